# revision 22
# baseline (speedup 1.0000x reference)
"""GNN message-passing layer (segment_sum + BatchNorm(train) + ReLU) on 8 Trainium2 cores.

Strategy (dst-sharded, fully local segment sum):
  - Sort edges by (dst_tile, src_half, src). dst tiles are 128-node windows;
    each core owns a contiguous block of tiles, so the segment-sum is local
    to one core (no [N,D] all-reduce at all).
  - Per dst tile: bulk-gather h[src] rows via the SWDGE dma_gather custom
    instruction (int16 indices => the node table is split at SPLIT=25000 into
    two <32768-row halves; chunks are homogeneous lo/hi by construction).
    Gathers are descriptor-rate-bound (~10ns/row, HW-measured), so rows are
    packed to 768B: hi plane bf16 + lo correction plane fp8(x64), giving
    ~fp32 accuracy at 75% of the bytes of a full hi/lo bf16 pair.
  - Segment sum via per-chunk [128e x 128n] 0/1 masks on the vector engine
    (mask = is_equal(iota_row, dst_local)) feeding PE matmuls that
    accumulate in fp32 PSUM:  agg = sum_e onehot(dst)*(hi[src] + lo[src]).
  - BatchNorm stats: per-tile ones-vector matmuls accumulate column sums of
    agg and agg^2 in PSUM; a tiny [1,512] AllReduce across the 8 cores gives
    global mean/var; the elementwise chain is local; output rows are written
    dst-sharded and concatenated on the host.
"""

import math
import os
import sys
from contextlib import ExitStack
from dataclasses import dataclass

import numpy as np

try:
    import ml_dtypes
except ImportError:  # pragma: no cover
    ml_dtypes = None

_REPO = "/opt/trn_rl_repo"
if _REPO not in sys.path and os.path.isdir(_REPO):
    sys.path.insert(0, _REPO)

P = 128
BN_EPS = 1e-5
LO_SCALE = 64.0  # lo plane stored as fp8e4m3 * LO_SCALE; mask carries 1/64
GSPLIT = 8  # max chunks per dma_gather piece (desc-gen/transfer pipelining)


def _pieces_cnt(cnt_lo, cnt_hi, c_lo, gsplit=GSPLIT):
    """(chunk0, nchunks, valid_count, half) gather pieces for one tile.

    Chunk columns [0, c_lo) hold lo-half rows, [c_lo, c) hi-half rows.
    valid_count is the number of real (non -1) indices in the piece; the
    remainder of the last chunk is -1 filled and skipped by the gather.
    """
    out = []
    for cnt, base, half in ((cnt_lo, 0, 0), (cnt_hi, c_lo, 1)):
        nch = math.ceil(cnt / P)
        a0 = 0
        while a0 < nch:
            a1 = min(a0 + gsplit, nch)
            valid = min(cnt - a0 * P, (a1 - a0) * P)
            out.append((base + a0, a1 - a0, valid, half))
            a0 = a1
    return out


class _nullcm:
    def __enter__(self):
        return None

    def __exit__(self, *a):
        return False


@dataclass(frozen=True)
class Cfg:
    n_nodes: int
    d: int
    n_cores: int
    split: int
    c_lo: int
    c_hi: int
    lo_mode: str = "fp8"  # "fp8" | "bf16" | "none"
    # per tile-rank valid gather counts (common across cores; tiles are
    # processed in per-core descending-count order so ranks align)
    lo_eff: tuple = ()
    hi_eff: tuple = ()

    @property
    def n_tiles(self) -> int:
        return math.ceil(self.n_nodes / P)

    @property
    def nt(self) -> int:  # tiles per core
        return math.ceil(self.n_tiles / self.n_cores)

    @property
    def c(self) -> int:
        return self.c_lo + self.c_hi

    @property
    def row_bytes(self) -> int:  # gathered bytes per node row
        return {"fp8": 3 * self.d, "bf16": 4 * self.d, "none": 2 * self.d}[
            self.lo_mode
        ]

    @property
    def g_bufs(self) -> int:
        return 3 if self.row_bytes <= 3 * self.d else 2


def _bf16(x):
    return x.astype(ml_dtypes.bfloat16)


def _pack_table(h, lo_mode):
    """Build the gather table. Returns (array, np_dtype_name)."""
    hi = _bf16(h)
    if lo_mode == "none":
        return np.ascontiguousarray(hi)
    lo = h - hi.astype(np.float32)
    if lo_mode == "bf16":
        return np.ascontiguousarray(np.concatenate([hi, _bf16(lo)], axis=1))
    # fp8: [hi bf16 bytes | fp8(lo*64) bytes] as one int8 row
    lo8 = (lo * LO_SCALE).astype(ml_dtypes.float8_e4m3)
    hi_b = hi.view(np.int8)  # [N, 2D]
    lo_b = lo8.view(np.int8)  # [N, D]
    return np.ascontiguousarray(np.concatenate([hi_b, lo_b], axis=1))


def prep_inputs(cfg_partial, h, gamma, beta, src, dst):
    """Host-side preprocessing. Returns (cfg, shared_arrays, per_core_arrays)."""
    n = cfg_partial["n_nodes"]
    d = cfg_partial["d"]
    n_cores = cfg_partial["n_cores"]
    split = cfg_partial["split"]
    lo_mode = cfg_partial.get("lo_mode", "fp8")

    src = np.asarray(src).astype(np.int64)
    dst = np.asarray(dst).astype(np.int64)
    h = np.asarray(h, dtype=np.float32)

    n_tiles = math.ceil(n / P)
    nt = math.ceil(n_tiles / n_cores)
    n_tiles_pad = nt * n_cores

    tile_id = dst // P
    local = (dst % P).astype(np.float32)
    is_hi = (src >= split).astype(np.int64)

    order = np.lexsort((src, is_hi, tile_id))
    st = src[order]
    lt = local[order]
    ht = is_hi[order]
    tid = tile_id[order]

    group = tid * 2 + ht
    counts = np.bincount(group, minlength=2 * n_tiles_pad)
    starts = np.zeros(2 * n_tiles_pad + 1, dtype=np.int64)
    np.cumsum(counts, out=starts[1:])
    pos = np.arange(len(st), dtype=np.int64) - np.repeat(starts[:-1], counts)
    counts2 = counts.reshape(n_tiles_pad, 2)

    # Per-core processing order: tiles sorted by descending total count so the
    # rank-r counts are nearly equal across cores; the SPMD program bakes the
    # per-rank max as its valid gather count and -1 pads (skipped by SWDGE)
    # fill the rest of the last chunk.
    tiles = np.arange(n_tiles_pad).reshape(n_cores, nt)
    tot = counts2.sum(1)
    perm = np.stack(
        [tiles[k][np.argsort(-tot[tiles[k]], kind="stable")] for k in range(n_cores)]
    )  # [n_cores, nt]
    lo_common = counts2[perm, 0].max(axis=0)  # [nt]
    hi_common = counts2[perm, 1].max(axis=0)
    c_lo = max(1, int(np.max(np.ceil(lo_common / P))))
    c_hi = max(1, int(np.max(np.ceil(hi_common / P))))

    lo_eff = np.maximum(lo_common, 1)
    hi_eff = np.maximum(hi_common, 1)

    cfg = Cfg(
        n_nodes=n, d=d, n_cores=n_cores, split=split, c_lo=c_lo, c_hi=c_hi,
        lo_mode=lo_mode,
        lo_eff=tuple(int(x) for x in lo_eff),
        hi_eff=tuple(int(x) for x in hi_eff),
    )
    c = cfg.c

    slot = np.where(ht == 1, cfg.c_lo * P + pos, pos)
    rng = np.random.default_rng(1234)
    idx_pad = np.full((n_tiles_pad, c * P), -1, dtype=np.int16)
    dst_pad = np.full((n_tiles_pad, c * P), -1.0, dtype=np.float32)
    idx_rel = (st - ht * split).astype(np.int16)
    idx_pad[tid, slot] = idx_rel
    dst_pad[tid, slot] = lt

    # Common-count pad slots get pseudo-random spread indices (a constant pad
    # index funnels every pad descriptor to one HBM channel; HW-measured 2.5x
    # slow). Slots beyond lo_eff/hi_eff stay -1 and cost no descriptor.
    n_lo, n_hi = split, n - split
    for k in range(n_cores):
        for r in range(nt):
            t = perm[k, r]
            cl = int(counts2[t, 0])
            if lo_eff[r] > cl:
                idx_pad[t, cl : lo_eff[r]] = rng.integers(
                    0, n_lo, lo_eff[r] - cl, dtype=np.int16
                )
            ch = int(counts2[t, 1])
            if hi_eff[r] > ch:
                idx_pad[t, c_lo * P + ch : c_lo * P + hi_eff[r]] = rng.integers(
                    0, n_hi, hi_eff[r] - ch, dtype=np.int16
                )

    h2 = _pack_table(h, lo_mode)

    iota = np.tile(np.arange(P, dtype=np.float32), (P, 1))
    gb = np.concatenate(
        [np.asarray(gamma, np.float32), np.asarray(beta, np.float32)]
    ).reshape(1, 2 * d)

    shared = dict(h2=h2, iota=iota, gb=gb)

    per_core = []
    for k in range(n_cores):
        ip = idx_pad[perm[k]]  # [nt, c*P] int16, processing order
        blk = ip.reshape(nt, c * 8, 16).transpose(0, 2, 1)  # [nt, 16, c*8]
        idx16 = np.tile(blk.transpose(1, 0, 2).reshape(16, nt * c * 8), (8, 1))
        dstv = (
            dst_pad[perm[k]]
            .reshape(nt, c, P)
            .transpose(2, 0, 1)
            .reshape(P, nt * c)
        )
        per_core.append(
            dict(
                idx16=np.ascontiguousarray(idx16),
                dstv=np.ascontiguousarray(dstv),
                tile_order=perm[k].copy(),
            )
        )
    return cfg, shared, per_core


def build_program(cfg: Cfg, repeat_phase1: int = 1, gather_split: int = GSPLIT,
                  g_bufs: int | None = None, nq: int = 4):
    import concourse.bacc as bacc
    import concourse.tile as tile
    from concourse import mybir

    dt = mybir.dt
    d = cfg.d
    nt = cfg.nt
    c_lo, c_hi, c = cfg.c_lo, cfg.c_hi, cfg.c
    rb = cfg.row_bytes  # bytes per table row

    tab_dt = {"fp8": dt.int8, "bf16": dt.bfloat16, "none": dt.bfloat16}[cfg.lo_mode]
    tab_cols = rb // mybir.dt.size(tab_dt)

    # 4 SWDGE queues: gather descriptor processing parallelizes across the
    # gpsimd SWDGE cores (HW-measured 9.1 -> 4.9 ns/row going 1q -> 4q).
    nc = bacc.Bacc(
        "TRN2", target_bir_lowering=False, debug=False, num_devices=cfg.n_cores,
        num_swdge_queues=nq,
    )

    h2_t = nc.dram_tensor("h2", [cfg.n_nodes, tab_cols], tab_dt, kind="ExternalInput")
    idx_t = nc.dram_tensor("idx16", [P, nt * c * 8], dt.int16, kind="ExternalInput")
    dstv_t = nc.dram_tensor("dstv", [P, nt * c], dt.float32, kind="ExternalInput")
    iota_t = nc.dram_tensor("iota", [P, P], dt.float32, kind="ExternalInput")
    gb_t = nc.dram_tensor("gb", [1, 2 * d], dt.float32, kind="ExternalInput")
    out_t = nc.dram_tensor("out", [nt * P, d], dt.float32, kind="ExternalOutput")

    h2_ap = h2_t.ap()
    h2_half = [h2_ap[0 : cfg.split, :], h2_ap[cfg.split : cfg.n_nodes, :]]

    def rhs_views(g, cc):
        """matmul rhs slices (list of (rhs_ap, which_mask)) for chunk cc."""
        row = g[:, cc, :]
        if cfg.lo_mode == "none":
            return [(row, "hi")]
        if cfg.lo_mode == "bf16":
            return [(row[:, 0:d], "hi"), (row[:, d : 2 * d], "hi")]
        return [
            (row[:, 0 : 2 * d].bitcast(dt.bfloat16), "hi"),
            (row[:, 2 * d : 3 * d].bitcast(dt.float8e4), "lo"),
        ]

    with tile.TileContext(nc) as tc, ExitStack() as ctx:
        singles = ctx.enter_context(tc.tile_pool(name="singles", bufs=1))
        if g_bufs is None:
            g_bufs = 3 if rb <= 3 * d else 2
        gpool = ctx.enter_context(tc.tile_pool(name="g", bufs=g_bufs))
        mpool = ctx.enter_context(tc.tile_pool(name="mk", bufs=12))
        spool = ctx.enter_context(tc.tile_pool(name="scr", bufs=3))
        pp = ctx.enter_context(tc.tile_pool(name="ps", bufs=2, space="PSUM"))
        pstat = ctx.enter_context(tc.tile_pool(name="pstat", bufs=1, space="PSUM"))
        dram = ctx.enter_context(tc.tile_pool(name="dram", bufs=2, space="DRAM"))

        idx_sb = singles.tile([P, nt * c * 8], dt.int16)
        nc.sync.dma_start(out=idx_sb[:], in_=idx_t.ap())
        dstv_sb = singles.tile([P, nt * c], dt.float32)
        nc.sync.dma_start(out=dstv_sb[:], in_=dstv_t.ap())
        iota_sb = singles.tile([P, P], dt.float32)
        nc.sync.dma_start(out=iota_sb[:], in_=iota_t.ap())
        gb_sb = singles.tile([1, 2 * d], dt.float32)
        nc.sync.dma_start(out=gb_sb[:], in_=gb_t.ap())

        ones_col = singles.tile([P, 1], dt.float32)
        nc.vector.memset(ones_col[:], 1.0)
        ones_row = singles.tile([1, P], dt.float32)
        nc.vector.memset(ones_row[:], 1.0)
        eps_sb = singles.tile([1, 1], dt.float32)
        nc.vector.memset(eps_sb[:], BN_EPS)

        agg = singles.tile([P, nt * d], dt.float32)
        psum_sum = pstat.tile([1, d], dt.float32)
        psum_sq = pstat.tile([1, d], dt.float32)

        # Tile assigns DMASW sem lanes to Pool-engine DMAs round-robin over 8
        # lanes in emission order, and the ucode locks each lane to the first
        # SWDGE queue that uses it -- so queue choice must be a pure function
        # of the lane. The two collective gpsimd.dma_starts after the gathers
        # are hardwired to queue 0, so their lanes map to 0; the remaining six
        # lanes spread over queues 1-3 (evenly: each lane sees 1/8 of pieces).
        # Staggered For_i loops rotate 5 lanes instead, so the slope
        # diagnostic build stays on queue 0.
        n_pieces_total = sum(
            len(_pieces_cnt(cfg.lo_eff[t], cfg.hi_eff[t], c_lo, gather_split))
            for t in range(nt)
        )
        lane_q = [0] * 8
        if repeat_phase1 == 1 and nq > 1:
            coll_lanes = {n_pieces_total % 8, (n_pieces_total + 1) % 8}
            others = [q % nq for q in range(1, 7)] if nq == 2 else [1, 2, 3, 1, 2, 3][: 6] if nq == 4 else [q % nq for q in range(6)]
            spread = [q if q != 0 or nq == 2 else 1 for q in others]
            for lane in range(8):
                if lane not in coll_lanes:
                    lane_q[lane] = spread.pop(0)
        pool_dma_ctr = [0]

        def next_q():
            q = lane_q[pool_dma_ctr[0] % 8]
            pool_dma_ctr[0] += 1
            return q

        rep_cm = tc.For_i(0, repeat_phase1, 1) if repeat_phase1 > 1 else _nullcm()
        with rep_cm:
          for t in range(nt):
            g = gpool.tile([P, c, tab_cols], tab_dt, tag="g")
            # split each half's gather into <=GSPLIT-chunk pieces: smaller
            # SWDGE ops pipeline desc-gen with the transfer drain. valid counts
            # (num_idxs_reg) stop descriptor generation at the -1 pad tail.
            # pre-zero each half's partial last chunk: the gather stops at
            # `valid` (the -1 pad tail emits no descriptors) but the masked
            # matmuls read all 128 partitions of that chunk. Program-order
            # WAW puts the gather's rows on top of the zeros.
            for eff, base in ((cfg.lo_eff[t], 0), (cfg.hi_eff[t], c_lo)):
                if eff % P:
                    nc.vector.memset(g[:, base + eff // P, :], 0)
            for c0, nck, valid, half in _pieces_cnt(
                cfg.lo_eff[t], cfg.hi_eff[t], c_lo, gather_split
            ):
                nc.gpsimd.dma_gather(
                    g[:, c0 : c0 + nck, :],
                    h2_half[half],
                    idx_sb[:, t * c * 8 + c0 * 8 : t * c * 8 + (c0 + nck) * 8],
                    nck * P,
                    valid,
                    tab_cols,
                    single_packet=False,
                    queue_num=next_q(),
                )
            fp8 = cfg.lo_mode == "fp8"
            ps = pp.tile([P, d], dt.float32, tag="ps")
            if fp8:
                ps_lo = pp.tile([P, d], dt.float32, tag="pslo")
            else:
                ps_lo = None
            chunk_list = list(range(math.ceil(cfg.lo_eff[t] / P))) + list(
                range(c_lo, c_lo + math.ceil(cfg.hi_eff[t] / P))
            )
            n_ch = len(chunk_list)
            for j, cc in enumerate(chunk_list):
                views = rhs_views(g, cc)
                mk_hi = mpool.tile([P, P], dt.bfloat16, tag="mkhi")
                nc.vector.tensor_scalar(
                    out=mk_hi[:],
                    in0=iota_sb[:],
                    scalar1=dstv_sb[:, t * c + cc : t * c + cc + 1],
                    scalar2=None,
                    op0=mybir.AluOpType.is_equal,
                )
                n_to_ps = sum(1 for _, w in views if not (fp8 and w == "lo"))
                j_ps = 0
                for rhs, which in views:
                    # lo plane accumulates in its own PSUM with the SAME bf16
                    # 0/1 mask (mixed-dtype matmul); the 1/LO_SCALE is applied
                    # once per tile at PSUM-combine time.
                    if fp8 and which == "lo":
                        nc.tensor.matmul(
                            ps_lo[:], mk_hi[:], rhs,
                            start=(j == 0), stop=(j == n_ch - 1),
                        )
                    else:
                        nc.tensor.matmul(
                            ps[:], mk_hi[:], rhs,
                            start=(j == 0 and j_ps == 0),
                            stop=(j == n_ch - 1 and j_ps == n_to_ps - 1),
                        )
                        j_ps += 1
            a = agg[:, t * d : (t + 1) * d]
            if fp8:
                lo_sc = spool.tile([P, d], dt.float32, tag="losc")
                nc.vector.tensor_scalar_mul(lo_sc[:], ps_lo[:], 1.0 / LO_SCALE)
                nc.vector.tensor_add(out=a, in0=lo_sc[:], in1=ps[:])
            else:
                nc.scalar.activation(a, ps[:], mybir.ActivationFunctionType.Copy)
            sq = spool.tile([P, d], dt.float32, tag="sq")
            nc.scalar.activation(sq[:], a, mybir.ActivationFunctionType.Square)
            nc.tensor.matmul(
                psum_sum[:], ones_col[:], a, start=(t == 0), stop=(t == nt - 1)
            )
            nc.tensor.matmul(
                psum_sq[:], ones_col[:], sq[:], start=(t == 0), stop=(t == nt - 1)
            )

        # ---- phase 2: global stats + scale/shift --------------------------
        stats = singles.tile([1, 2 * d], dt.float32)
        nc.vector.tensor_copy(out=stats[:, 0:d], in_=psum_sum[:])
        nc.vector.tensor_copy(out=stats[:, d : 2 * d], in_=psum_sq[:])

        cin = dram.tile([1, 2 * d], dt.float32)
        cout = dram.tile([1, 2 * d], dt.float32)
        nc.gpsimd.dma_start(out=cin[:], in_=stats[:])
        nc.gpsimd.collective_compute(
            "AllReduce",
            mybir.AluOpType.add,
            replica_groups=[list(range(cfg.n_cores))],
            ins=[cin.opt()],
            outs=[cout.opt()],
        )
        nc.gpsimd.dma_start(out=stats[:], in_=cout[:])

        inv_n = 1.0 / float(cfg.n_nodes)
        mean = singles.tile([1, d], dt.float32)
        ex2 = singles.tile([1, d], dt.float32)
        nc.vector.tensor_scalar_mul(mean[:], stats[:, 0:d], inv_n)
        nc.vector.tensor_scalar_mul(ex2[:], stats[:, d : 2 * d], inv_n)
        var = singles.tile([1, d], dt.float32)
        nc.vector.tensor_mul(var[:], mean[:], mean[:])
        nc.vector.tensor_tensor(
            out=var[:], in0=ex2[:], in1=var[:], op=mybir.AluOpType.subtract
        )
        rstd = singles.tile([1, d], dt.float32)
        nc.scalar.activation(
            rstd[:],
            var[:],
            mybir.ActivationFunctionType.Sqrt,
            bias=eps_sb[:],
            scale=1.0,
        )
        nc.vector.reciprocal(out=rstd[:], in_=rstd[:])

        scsh = singles.tile([1, 2 * d], dt.float32)
        nc.vector.tensor_mul(scsh[:, 0:d], gb_sb[:, 0:d], rstd[:])  # scale
        tmp = singles.tile([1, d], dt.float32)
        nc.vector.tensor_mul(tmp[:], mean[:], scsh[:, 0:d])
        nc.vector.tensor_tensor(
            out=scsh[:, d : 2 * d],
            in0=gb_sb[:, d : 2 * d],
            in1=tmp[:],
            op=mybir.AluOpType.subtract,
        )

        psb = pstat.tile([P, 2 * d], dt.float32)
        nc.tensor.matmul(psb[:], ones_row[:], scsh[:], start=True, stop=True)
        bc = singles.tile([P, 2 * d], dt.float32)
        nc.vector.tensor_copy(out=bc[:], in_=psb[:])

        # ---- phase 3: normalize + relu + writeback ------------------------
        out_ap = out_t.ap()
        for t in range(nt):
            a = agg[:, t * d : (t + 1) * d]
            y = spool.tile([P, d], dt.float32, tag="y")
            nc.vector.tensor_mul(y[:], a, bc[:, 0:d])
            nc.vector.tensor_add(out=y[:], in0=y[:], in1=bc[:, d : 2 * d])
            nc.vector.tensor_scalar_max(y[:], y[:], 0.0)
            nc.sync.dma_start(out=out_ap[t * P : (t + 1) * P, :], in_=y[:])

    nc.compile()
    return nc


_CACHE: dict = {}


def _get_program(cfg: Cfg):
    if cfg not in _CACHE:
        _CACHE[cfg] = build_program(cfg)
    return _CACHE[cfg]


def run(cfg: Cfg, shared, per_core, trace=False):
    from concourse.bass_utils import run_bass_kernel_spmd

    nc = _get_program(cfg)
    in_maps = [
        dict(
            h2=shared["h2"],
            idx16=pc["idx16"],
            dstv=pc["dstv"],
            iota=shared["iota"],
            gb=shared["gb"],
        )
        for pc in per_core
    ]
    res = run_bass_kernel_spmd(
        nc, in_maps, core_ids=list(range(cfg.n_cores)), trace=trace
    )
    full = np.empty((cfg.nt * cfg.n_cores * P, cfg.d), np.float32)
    for k, r in enumerate(res.results):
        out_k = r["out"]
        for rank, t in enumerate(per_core[k]["tile_order"]):
            full[t * P : (t + 1) * P] = out_k[rank * P : (rank + 1) * P]
    return full[: cfg.n_nodes], res


def kernel(**inputs) -> np.ndarray:
    h = np.asarray(inputs["h"], dtype=np.float32)
    gamma = np.asarray(inputs["gamma"], dtype=np.float32)
    beta = np.asarray(inputs["beta"], dtype=np.float32)
    src = np.asarray(inputs["src"])
    dst = np.asarray(inputs["dst"])

    n, d = h.shape
    cfg_partial = dict(
        n_nodes=n, d=d, n_cores=8, split=min(n, 25000), lo_mode="none"
    )
    cfg, shared, per_core = prep_inputs(cfg_partial, h, gamma, beta, src, dst)
    full, _ = run(cfg, shared, per_core)
    return full.astype(np.float32)



# revision 23
# speedup vs baseline: 1.2114x; 1.2114x over previous
"""GNN message-passing layer (segment_sum + BatchNorm(train) + ReLU) on 8 Trainium2 cores.

Strategy (dst-sharded, fully local segment sum):
  - Sort edges by (dst_tile, src_half, src). dst tiles are 128-node windows;
    each core owns a fixed set of tiles, so the segment-sum is local to one
    core (no [N,D] all-reduce at all). Each core processes its tiles in
    descending-edge-count order so the rank-r tile's edge count is nearly
    equal across cores; the shared SPMD program bakes the per-rank max as
    its gather count (pads ~2%, with -1 index tails skipped by SWDGE).
  - Per dst tile: bulk-gather h[src] rows via the SWDGE dma_gather custom
    instruction (int16 indices => the node table is split at SPLIT=25000 into
    two <32768-row halves; chunks are homogeneous lo/hi by construction).
    Gathers cost ~9.4 ns/row on one SWDGE queue regardless of source
    (HBM or SBUF) or row bytes -- per-descriptor machinery bound. Spreading
    pieces over 4 SWDGE queues (lane-consistent with Tile's 8 DMASW sem
    lanes) reaches ~4.9 ns/row. Rows are bf16 (512B); hi/lo fp8 packing is
    not worth extra bytes at rel-err tolerance 2e-2 (bf16 gives ~2e-3).
  - Segment sum via per-chunk [128e x 128n] 0/1 masks on the vector engine
    (mask = is_equal(iota_row, dst_local)) feeding PE matmuls that
    accumulate in fp32 PSUM:  agg = sum_e onehot(dst) * h_bf16[src].
  - BatchNorm stats: per-tile ones-vector matmuls accumulate column sums of
    agg and agg^2 in PSUM; a tiny [1,512] AllReduce across the 8 cores gives
    global mean/var; the elementwise chain is local; output rows are written
    per-core and reassembled (tile permutation undone) on the host.
"""

import math
import os
import sys
from contextlib import ExitStack
from dataclasses import dataclass

import numpy as np

try:
    import ml_dtypes
except ImportError:  # pragma: no cover
    ml_dtypes = None

_REPO = "/opt/trn_rl_repo"
if _REPO not in sys.path and os.path.isdir(_REPO):
    sys.path.insert(0, _REPO)

P = 128
BN_EPS = 1e-5
LO_SCALE = 64.0  # lo plane stored as fp8e4m3 * LO_SCALE; mask carries 1/64
GSPLIT = 8  # max chunks per dma_gather piece (desc-gen/transfer pipelining)


def _pieces_cnt(cnt_lo, cnt_hi, c_lo, gsplit=GSPLIT):
    """(chunk0, nchunks, valid_count, half) gather pieces for one tile.

    Chunk columns [0, c_lo) hold lo-half rows, [c_lo, c) hi-half rows.
    valid_count is the number of real (non -1) indices in the piece; the
    remainder of the last chunk is -1 filled and skipped by the gather.
    """
    out = []
    for cnt, base, half in ((cnt_lo, 0, 0), (cnt_hi, c_lo, 1)):
        nch = math.ceil(cnt / P)
        a0 = 0
        while a0 < nch:
            a1 = min(a0 + gsplit, nch)
            valid = min(cnt - a0 * P, (a1 - a0) * P)
            out.append((base + a0, a1 - a0, valid, half))
            a0 = a1
    return out


class _nullcm:
    def __enter__(self):
        return None

    def __exit__(self, *a):
        return False


@dataclass(frozen=True)
class Cfg:
    n_nodes: int
    d: int
    n_cores: int
    split: int
    c_lo: int
    c_hi: int
    lo_mode: str = "fp8"  # "fp8" | "bf16" | "none"
    # per tile-rank valid gather counts (common across cores; tiles are
    # processed in per-core descending-count order so ranks align)
    lo_eff: tuple = ()
    hi_eff: tuple = ()

    @property
    def n_tiles(self) -> int:
        return math.ceil(self.n_nodes / P)

    @property
    def nt(self) -> int:  # tiles per core
        return math.ceil(self.n_tiles / self.n_cores)

    @property
    def c(self) -> int:
        return self.c_lo + self.c_hi

    @property
    def row_bytes(self) -> int:  # gathered bytes per node row
        return {"fp8": 3 * self.d, "bf16": 4 * self.d, "none": 2 * self.d}[
            self.lo_mode
        ]

    @property
    def g_bufs(self) -> int:
        return 3 if self.row_bytes <= 3 * self.d else 2


def _bf16(x):
    return x.astype(ml_dtypes.bfloat16)


def _pack_table(h, lo_mode):
    """Build the gather table. Returns (array, np_dtype_name)."""
    hi = _bf16(h)
    if lo_mode == "none":
        return np.ascontiguousarray(hi)
    lo = h - hi.astype(np.float32)
    if lo_mode == "bf16":
        return np.ascontiguousarray(np.concatenate([hi, _bf16(lo)], axis=1))
    # fp8: [hi bf16 bytes | fp8(lo*64) bytes] as one int8 row
    lo8 = (lo * LO_SCALE).astype(ml_dtypes.float8_e4m3)
    hi_b = hi.view(np.int8)  # [N, 2D]
    lo_b = lo8.view(np.int8)  # [N, D]
    return np.ascontiguousarray(np.concatenate([hi_b, lo_b], axis=1))


def prep_inputs(cfg_partial, h, gamma, beta, src, dst):
    """Host-side preprocessing. Returns (cfg, shared_arrays, per_core_arrays)."""
    n = cfg_partial["n_nodes"]
    d = cfg_partial["d"]
    n_cores = cfg_partial["n_cores"]
    split = cfg_partial["split"]
    lo_mode = cfg_partial.get("lo_mode", "fp8")

    src = np.asarray(src).astype(np.int64)
    dst = np.asarray(dst).astype(np.int64)
    h = np.asarray(h, dtype=np.float32)

    n_tiles = math.ceil(n / P)
    nt = math.ceil(n_tiles / n_cores)
    n_tiles_pad = nt * n_cores

    tile_id = dst // P
    local = (dst % P).astype(np.float32)
    is_hi = (src >= split).astype(np.int64)

    order = np.lexsort((src, is_hi, tile_id))
    st = src[order]
    lt = local[order]
    ht = is_hi[order]
    tid = tile_id[order]

    group = tid * 2 + ht
    counts = np.bincount(group, minlength=2 * n_tiles_pad)
    starts = np.zeros(2 * n_tiles_pad + 1, dtype=np.int64)
    np.cumsum(counts, out=starts[1:])
    pos = np.arange(len(st), dtype=np.int64) - np.repeat(starts[:-1], counts)
    counts2 = counts.reshape(n_tiles_pad, 2)

    # Per-core processing order: tiles sorted by descending total count so the
    # rank-r counts are nearly equal across cores; the SPMD program bakes the
    # per-rank max as its valid gather count and -1 pads (skipped by SWDGE)
    # fill the rest of the last chunk.
    tiles = np.arange(n_tiles_pad).reshape(n_cores, nt)
    tot = counts2.sum(1)
    perm = np.stack(
        [tiles[k][np.argsort(-tot[tiles[k]], kind="stable")] for k in range(n_cores)]
    )  # [n_cores, nt]
    lo_common = counts2[perm, 0].max(axis=0)  # [nt]
    hi_common = counts2[perm, 1].max(axis=0)
    c_lo = max(1, int(np.max(np.ceil(lo_common / P))))
    c_hi = max(1, int(np.max(np.ceil(hi_common / P))))

    lo_eff = np.maximum(lo_common, 1)
    hi_eff = np.maximum(hi_common, 1)

    cfg = Cfg(
        n_nodes=n, d=d, n_cores=n_cores, split=split, c_lo=c_lo, c_hi=c_hi,
        lo_mode=lo_mode,
        lo_eff=tuple(int(x) for x in lo_eff),
        hi_eff=tuple(int(x) for x in hi_eff),
    )
    c = cfg.c

    slot = np.where(ht == 1, cfg.c_lo * P + pos, pos)
    rng = np.random.default_rng(1234)
    idx_pad = np.full((n_tiles_pad, c * P), -1, dtype=np.int16)
    dst_pad = np.full((n_tiles_pad, c * P), -1.0, dtype=np.float32)
    idx_rel = (st - ht * split).astype(np.int16)
    idx_pad[tid, slot] = idx_rel
    dst_pad[tid, slot] = lt

    # Common-count pad slots get pseudo-random spread indices (a constant pad
    # index funnels every pad descriptor to one HBM channel; HW-measured 2.5x
    # slow). Slots beyond lo_eff/hi_eff stay -1 and cost no descriptor.
    n_lo, n_hi = split, n - split
    for k in range(n_cores):
        for r in range(nt):
            t = perm[k, r]
            cl = int(counts2[t, 0])
            if lo_eff[r] > cl:
                idx_pad[t, cl : lo_eff[r]] = rng.integers(
                    0, n_lo, lo_eff[r] - cl, dtype=np.int16
                )
            ch = int(counts2[t, 1])
            if hi_eff[r] > ch:
                idx_pad[t, c_lo * P + ch : c_lo * P + hi_eff[r]] = rng.integers(
                    0, n_hi, hi_eff[r] - ch, dtype=np.int16
                )

    h2 = _pack_table(h, lo_mode)

    iota = np.tile(np.arange(P, dtype=np.float32), (P, 1))
    gb = np.concatenate(
        [np.asarray(gamma, np.float32), np.asarray(beta, np.float32)]
    ).reshape(1, 2 * d)

    shared = dict(h2=h2, iota=iota, gb=gb)

    per_core = []
    for k in range(n_cores):
        ip = idx_pad[perm[k]]  # [nt, c*P] int16, processing order
        blk = ip.reshape(nt, c * 8, 16).transpose(0, 2, 1)  # [nt, 16, c*8]
        idx16 = np.tile(blk.transpose(1, 0, 2).reshape(16, nt * c * 8), (8, 1))
        dstv = (
            dst_pad[perm[k]]
            .reshape(nt, c, P)
            .transpose(2, 0, 1)
            .reshape(P, nt * c)
        )
        per_core.append(
            dict(
                idx16=np.ascontiguousarray(idx16),
                dstv=np.ascontiguousarray(dstv),
                tile_order=perm[k].copy(),
            )
        )
    return cfg, shared, per_core


def build_program(cfg: Cfg, repeat_phase1: int = 1, gather_split: int = GSPLIT,
                  g_bufs: int | None = None, nq: int = 4):
    import concourse.bacc as bacc
    import concourse.tile as tile
    from concourse import mybir

    dt = mybir.dt
    d = cfg.d
    nt = cfg.nt
    c_lo, c_hi, c = cfg.c_lo, cfg.c_hi, cfg.c
    rb = cfg.row_bytes  # bytes per table row

    tab_dt = {"fp8": dt.int8, "bf16": dt.bfloat16, "none": dt.bfloat16}[cfg.lo_mode]
    tab_cols = rb // mybir.dt.size(tab_dt)

    # 4 SWDGE queues: gather descriptor processing parallelizes across the
    # gpsimd SWDGE cores (HW-measured 9.1 -> 4.9 ns/row going 1q -> 4q).
    nc = bacc.Bacc(
        "TRN2", target_bir_lowering=False, debug=False, num_devices=cfg.n_cores,
        num_swdge_queues=nq,
    )

    h2_t = nc.dram_tensor("h2", [cfg.n_nodes, tab_cols], tab_dt, kind="ExternalInput")
    idx_t = nc.dram_tensor("idx16", [P, nt * c * 8], dt.int16, kind="ExternalInput")
    dstv_t = nc.dram_tensor("dstv", [P, nt * c], dt.float32, kind="ExternalInput")
    iota_t = nc.dram_tensor("iota", [P, P], dt.float32, kind="ExternalInput")
    gb_t = nc.dram_tensor("gb", [1, 2 * d], dt.float32, kind="ExternalInput")
    out_t = nc.dram_tensor("out", [nt * P, d], dt.float32, kind="ExternalOutput")

    h2_ap = h2_t.ap()
    h2_half = [h2_ap[0 : cfg.split, :], h2_ap[cfg.split : cfg.n_nodes, :]]

    def rhs_views(g, cc):
        """matmul rhs slices (list of (rhs_ap, which_mask)) for chunk cc."""
        row = g[:, cc, :]
        if cfg.lo_mode == "none":
            return [(row, "hi")]
        if cfg.lo_mode == "bf16":
            return [(row[:, 0:d], "hi"), (row[:, d : 2 * d], "hi")]
        return [
            (row[:, 0 : 2 * d].bitcast(dt.bfloat16), "hi"),
            (row[:, 2 * d : 3 * d].bitcast(dt.float8e4), "lo"),
        ]

    with tile.TileContext(nc) as tc, ExitStack() as ctx:
        singles = ctx.enter_context(tc.tile_pool(name="singles", bufs=1))
        if g_bufs is None:
            g_bufs = 3 if rb <= 3 * d else 2
        gpool = ctx.enter_context(tc.tile_pool(name="g", bufs=g_bufs))
        mpool = ctx.enter_context(tc.tile_pool(name="mk", bufs=12))
        spool = ctx.enter_context(tc.tile_pool(name="scr", bufs=3))
        pp = ctx.enter_context(tc.tile_pool(name="ps", bufs=2, space="PSUM"))
        pstat = ctx.enter_context(tc.tile_pool(name="pstat", bufs=1, space="PSUM"))
        dram = ctx.enter_context(tc.tile_pool(name="dram", bufs=2, space="DRAM"))

        idx_sb = singles.tile([P, nt * c * 8], dt.int16)
        nc.sync.dma_start(out=idx_sb[:], in_=idx_t.ap())
        dstv_sb = singles.tile([P, nt * c], dt.float32)
        nc.sync.dma_start(out=dstv_sb[:], in_=dstv_t.ap())
        iota_sb = singles.tile([P, P], dt.float32)
        nc.sync.dma_start(out=iota_sb[:], in_=iota_t.ap())
        gb_sb = singles.tile([1, 2 * d], dt.float32)
        nc.sync.dma_start(out=gb_sb[:], in_=gb_t.ap())

        ones_col = singles.tile([P, 1], dt.float32)
        nc.vector.memset(ones_col[:], 1.0)
        ones_row = singles.tile([1, P], dt.float32)
        nc.vector.memset(ones_row[:], 1.0)
        eps_sb = singles.tile([1, 1], dt.float32)
        nc.vector.memset(eps_sb[:], BN_EPS)

        agg = singles.tile([P, nt * d], dt.float32)
        psum_sum = pstat.tile([1, d], dt.float32)
        psum_sq = pstat.tile([1, d], dt.float32)

        # Tile assigns DMASW sem lanes to Pool-engine DMAs round-robin over 8
        # lanes in emission order, and the ucode locks each lane to the first
        # SWDGE queue that uses it -- so queue choice must be a pure function
        # of the lane. The two collective gpsimd.dma_starts after the gathers
        # are hardwired to queue 0, so their lanes map to 0; the remaining six
        # lanes spread over queues 1-3 (evenly: each lane sees 1/8 of pieces).
        # Staggered For_i loops rotate 5 lanes instead, so the slope
        # diagnostic build stays on queue 0.
        n_pieces_total = sum(
            len(_pieces_cnt(cfg.lo_eff[t], cfg.hi_eff[t], c_lo, gather_split))
            for t in range(nt)
        )
        lane_q = [0] * 8
        if repeat_phase1 == 1 and nq > 1:
            coll_lanes = {n_pieces_total % 8, (n_pieces_total + 1) % 8}
            others = [q % nq for q in range(1, 7)] if nq == 2 else [1, 2, 3, 1, 2, 3][: 6] if nq == 4 else [q % nq for q in range(6)]
            spread = [q if q != 0 or nq == 2 else 1 for q in others]
            for lane in range(8):
                if lane not in coll_lanes:
                    lane_q[lane] = spread.pop(0)
        pool_dma_ctr = [0]

        def next_q():
            q = lane_q[pool_dma_ctr[0] % 8]
            pool_dma_ctr[0] += 1
            return q

        rep_cm = tc.For_i(0, repeat_phase1, 1) if repeat_phase1 > 1 else _nullcm()
        with rep_cm:
          for t in range(nt):
            g = gpool.tile([P, c, tab_cols], tab_dt, tag="g")
            # split each half's gather into <=GSPLIT-chunk pieces: smaller
            # SWDGE ops pipeline desc-gen with the transfer drain. valid counts
            # (num_idxs_reg) stop descriptor generation at the -1 pad tail.
            # pre-zero each half's partial last chunk: the gather stops at
            # `valid` (the -1 pad tail emits no descriptors) but the masked
            # matmuls read all 128 partitions of that chunk. Program-order
            # WAW puts the gather's rows on top of the zeros.
            for eff, base in ((cfg.lo_eff[t], 0), (cfg.hi_eff[t], c_lo)):
                if eff % P:
                    nc.vector.memset(g[:, base + eff // P, :], 0)
            for c0, nck, valid, half in _pieces_cnt(
                cfg.lo_eff[t], cfg.hi_eff[t], c_lo, gather_split
            ):
                nc.gpsimd.dma_gather(
                    g[:, c0 : c0 + nck, :],
                    h2_half[half],
                    idx_sb[:, t * c * 8 + c0 * 8 : t * c * 8 + (c0 + nck) * 8],
                    nck * P,
                    valid,
                    tab_cols,
                    single_packet=False,
                    queue_num=next_q(),
                )
            fp8 = cfg.lo_mode == "fp8"
            ps = pp.tile([P, d], dt.float32, tag="ps")
            if fp8:
                ps_lo = pp.tile([P, d], dt.float32, tag="pslo")
            else:
                ps_lo = None
            chunk_list = list(range(math.ceil(cfg.lo_eff[t] / P))) + list(
                range(c_lo, c_lo + math.ceil(cfg.hi_eff[t] / P))
            )
            n_ch = len(chunk_list)
            for j, cc in enumerate(chunk_list):
                views = rhs_views(g, cc)
                mk_hi = mpool.tile([P, P], dt.bfloat16, tag="mkhi")
                nc.vector.tensor_scalar(
                    out=mk_hi[:],
                    in0=iota_sb[:],
                    scalar1=dstv_sb[:, t * c + cc : t * c + cc + 1],
                    scalar2=None,
                    op0=mybir.AluOpType.is_equal,
                )
                n_to_ps = sum(1 for _, w in views if not (fp8 and w == "lo"))
                j_ps = 0
                for rhs, which in views:
                    # lo plane accumulates in its own PSUM with the SAME bf16
                    # 0/1 mask (mixed-dtype matmul); the 1/LO_SCALE is applied
                    # once per tile at PSUM-combine time.
                    if fp8 and which == "lo":
                        nc.tensor.matmul(
                            ps_lo[:], mk_hi[:], rhs,
                            start=(j == 0), stop=(j == n_ch - 1),
                        )
                    else:
                        nc.tensor.matmul(
                            ps[:], mk_hi[:], rhs,
                            start=(j == 0 and j_ps == 0),
                            stop=(j == n_ch - 1 and j_ps == n_to_ps - 1),
                        )
                        j_ps += 1
            a = agg[:, t * d : (t + 1) * d]
            if fp8:
                lo_sc = spool.tile([P, d], dt.float32, tag="losc")
                nc.vector.tensor_scalar_mul(lo_sc[:], ps_lo[:], 1.0 / LO_SCALE)
                nc.vector.tensor_add(out=a, in0=lo_sc[:], in1=ps[:])
            else:
                nc.scalar.activation(a, ps[:], mybir.ActivationFunctionType.Copy)
            sq = spool.tile([P, d], dt.float32, tag="sq")
            nc.scalar.activation(sq[:], a, mybir.ActivationFunctionType.Square)
            nc.tensor.matmul(
                psum_sum[:], ones_col[:], a, start=(t == 0), stop=(t == nt - 1)
            )
            nc.tensor.matmul(
                psum_sq[:], ones_col[:], sq[:], start=(t == 0), stop=(t == nt - 1)
            )

        # ---- phase 2: global stats + scale/shift --------------------------
        stats = singles.tile([1, 2 * d], dt.float32)
        nc.vector.tensor_copy(out=stats[:, 0:d], in_=psum_sum[:])
        nc.vector.tensor_copy(out=stats[:, d : 2 * d], in_=psum_sq[:])

        cin = dram.tile([1, 2 * d], dt.float32)
        cout = dram.tile([1, 2 * d], dt.float32)
        nc.gpsimd.dma_start(out=cin[:], in_=stats[:])
        nc.gpsimd.collective_compute(
            "AllReduce",
            mybir.AluOpType.add,
            replica_groups=[list(range(cfg.n_cores))],
            ins=[cin.opt()],
            outs=[cout.opt()],
        )
        nc.gpsimd.dma_start(out=stats[:], in_=cout[:])

        inv_n = 1.0 / float(cfg.n_nodes)
        mean = singles.tile([1, d], dt.float32)
        ex2 = singles.tile([1, d], dt.float32)
        nc.vector.tensor_scalar_mul(mean[:], stats[:, 0:d], inv_n)
        nc.vector.tensor_scalar_mul(ex2[:], stats[:, d : 2 * d], inv_n)
        var = singles.tile([1, d], dt.float32)
        nc.vector.tensor_mul(var[:], mean[:], mean[:])
        nc.vector.tensor_tensor(
            out=var[:], in0=ex2[:], in1=var[:], op=mybir.AluOpType.subtract
        )
        rstd = singles.tile([1, d], dt.float32)
        nc.scalar.activation(
            rstd[:],
            var[:],
            mybir.ActivationFunctionType.Sqrt,
            bias=eps_sb[:],
            scale=1.0,
        )
        nc.vector.reciprocal(out=rstd[:], in_=rstd[:])

        scsh = singles.tile([1, 2 * d], dt.float32)
        nc.vector.tensor_mul(scsh[:, 0:d], gb_sb[:, 0:d], rstd[:])  # scale
        tmp = singles.tile([1, d], dt.float32)
        nc.vector.tensor_mul(tmp[:], mean[:], scsh[:, 0:d])
        nc.vector.tensor_tensor(
            out=scsh[:, d : 2 * d],
            in0=gb_sb[:, d : 2 * d],
            in1=tmp[:],
            op=mybir.AluOpType.subtract,
        )

        psb = pstat.tile([P, 2 * d], dt.float32)
        nc.tensor.matmul(psb[:], ones_row[:], scsh[:], start=True, stop=True)
        bc = singles.tile([P, 2 * d], dt.float32)
        nc.vector.tensor_copy(out=bc[:], in_=psb[:])

        # ---- phase 3: normalize + relu + writeback ------------------------
        out_ap = out_t.ap()
        for t in range(nt):
            a = agg[:, t * d : (t + 1) * d]
            y = spool.tile([P, d], dt.float32, tag="y")
            nc.vector.tensor_mul(y[:], a, bc[:, 0:d])
            nc.vector.tensor_add(out=y[:], in0=y[:], in1=bc[:, d : 2 * d])
            nc.vector.tensor_scalar_max(y[:], y[:], 0.0)
            nc.sync.dma_start(out=out_ap[t * P : (t + 1) * P, :], in_=y[:])

    nc.compile()
    return nc


_CACHE: dict = {}


def _get_program(cfg: Cfg):
    if cfg not in _CACHE:
        _CACHE[cfg] = build_program(cfg)
    return _CACHE[cfg]


def run(cfg: Cfg, shared, per_core, trace=False):
    from concourse.bass_utils import run_bass_kernel_spmd

    nc = _get_program(cfg)
    in_maps = [
        dict(
            h2=shared["h2"],
            idx16=pc["idx16"],
            dstv=pc["dstv"],
            iota=shared["iota"],
            gb=shared["gb"],
        )
        for pc in per_core
    ]
    res = run_bass_kernel_spmd(
        nc, in_maps, core_ids=list(range(cfg.n_cores)), trace=trace
    )
    full = np.empty((cfg.nt * cfg.n_cores * P, cfg.d), np.float32)
    for k, r in enumerate(res.results):
        out_k = r["out"]
        for rank, t in enumerate(per_core[k]["tile_order"]):
            full[t * P : (t + 1) * P] = out_k[rank * P : (rank + 1) * P]
    return full[: cfg.n_nodes], res


def kernel(**inputs) -> np.ndarray:
    h = np.asarray(inputs["h"], dtype=np.float32)
    gamma = np.asarray(inputs["gamma"], dtype=np.float32)
    beta = np.asarray(inputs["beta"], dtype=np.float32)
    src = np.asarray(inputs["src"])
    dst = np.asarray(inputs["dst"])

    n, d = h.shape
    cfg_partial = dict(
        n_nodes=n, d=d, n_cores=8, split=min(n, 25000), lo_mode="none"
    )
    cfg, shared, per_core = prep_inputs(cfg_partial, h, gamma, beta, src, dst)
    full, _ = run(cfg, shared, per_core)
    return full.astype(np.float32)



# revision 24
# speedup vs baseline: 1.2932x; 1.0675x over previous
"""GNN message-passing layer (segment_sum + BatchNorm(train) + ReLU) on 8 Trainium2 cores.

Strategy (dst-sharded, fully local segment sum):
  - Sort edges by (dst_tile, src_half, src). dst tiles are 128-node windows;
    each core owns a fixed set of tiles, so the segment-sum is local to one
    core (no [N,D] all-reduce at all). Each core processes its tiles in
    descending-edge-count order so the rank-r tile's edge count is nearly
    equal across cores; the shared SPMD program bakes the per-rank max as
    its gather count (pads ~2%, with -1 index tails skipped by SWDGE).
  - Per dst tile: bulk-gather h[src] rows via the SWDGE dma_gather custom
    instruction (int16 indices => the node table is split at SPLIT=25000 into
    two <32768-row halves; chunks are homogeneous lo/hi by construction).
    Gathers cost ~9.4 ns/row on one SWDGE queue regardless of source
    (HBM or SBUF) or row bytes -- per-descriptor machinery bound. Spreading
    pieces over 4 SWDGE queues (lane-consistent with Tile's 8 DMASW sem
    lanes) reaches ~4.9 ns/row. Rows are bf16 (512B); hi/lo fp8 packing is
    not worth extra bytes at rel-err tolerance 2e-2 (bf16 gives ~2e-3).
  - Segment sum via per-chunk [128e x 128n] 0/1 masks on the vector engine
    (mask = is_equal(iota_row, dst_local)) feeding PE matmuls that
    accumulate in fp32 PSUM:  agg = sum_e onehot(dst) * h_bf16[src].
  - BatchNorm stats: per-tile ones-vector matmuls accumulate column sums of
    agg and agg^2 in PSUM; a tiny [1,512] AllReduce across the 8 cores gives
    global mean/var; the elementwise chain is local; output rows are written
    per-core and reassembled (tile permutation undone) on the host.
"""

import math
import os
import sys
from contextlib import ExitStack
from dataclasses import dataclass

import numpy as np

try:
    import ml_dtypes
except ImportError:  # pragma: no cover
    ml_dtypes = None

_REPO = "/opt/trn_rl_repo"
if _REPO not in sys.path and os.path.isdir(_REPO):
    sys.path.insert(0, _REPO)

P = 128
BN_EPS = 1e-5
LO_SCALE = 64.0  # lo plane stored as fp8e4m3 * LO_SCALE; mask carries 1/64
GSPLIT = 8  # max chunks per dma_gather piece (desc-gen/transfer pipelining)


def _pieces_cnt(cnt_lo, cnt_hi, c_lo, gsplit=GSPLIT):
    """(chunk0, nchunks, valid_count, half) gather pieces for one tile.

    Chunk columns [0, c_lo) hold lo-half rows, [c_lo, c) hi-half rows.
    valid_count is the number of real (non -1) indices in the piece; the
    remainder of the last chunk is -1 filled and skipped by the gather.
    """
    out = []
    for cnt, base, half in ((cnt_lo, 0, 0), (cnt_hi, c_lo, 1)):
        nch = math.ceil(cnt / P)
        a0 = 0
        while a0 < nch:
            a1 = min(a0 + gsplit, nch)
            valid = min(cnt - a0 * P, (a1 - a0) * P)
            out.append((base + a0, a1 - a0, valid, half))
            a0 = a1
    return out


class _nullcm:
    def __enter__(self):
        return None

    def __exit__(self, *a):
        return False


@dataclass(frozen=True)
class Cfg:
    n_nodes: int
    d: int
    n_cores: int
    split: int
    c_lo: int
    c_hi: int
    lo_mode: str = "fp8"  # "fp8" | "bf16" | "none"
    # per tile-rank valid gather counts (common across cores; tiles are
    # processed in per-core descending-count order so ranks align)
    lo_eff: tuple = ()
    hi_eff: tuple = ()

    @property
    def n_tiles(self) -> int:
        return math.ceil(self.n_nodes / P)

    @property
    def nt(self) -> int:  # tiles per core
        return math.ceil(self.n_tiles / self.n_cores)

    @property
    def c(self) -> int:
        return self.c_lo + self.c_hi

    @property
    def row_bytes(self) -> int:  # gathered bytes per node row
        return {"fp8": 3 * self.d, "bf16": 4 * self.d, "none": 2 * self.d}[
            self.lo_mode
        ]

    @property
    def g_bufs(self) -> int:
        return 3 if self.row_bytes <= 3 * self.d else 2


def _bf16(x):
    return x.astype(ml_dtypes.bfloat16)


def _pack_table(h, lo_mode):
    """Build the gather table. Returns (array, np_dtype_name)."""
    hi = _bf16(h)
    if lo_mode == "none":
        return np.ascontiguousarray(hi)
    lo = h - hi.astype(np.float32)
    if lo_mode == "bf16":
        return np.ascontiguousarray(np.concatenate([hi, _bf16(lo)], axis=1))
    # fp8: [hi bf16 bytes | fp8(lo*64) bytes] as one int8 row
    lo8 = (lo * LO_SCALE).astype(ml_dtypes.float8_e4m3)
    hi_b = hi.view(np.int8)  # [N, 2D]
    lo_b = lo8.view(np.int8)  # [N, D]
    return np.ascontiguousarray(np.concatenate([hi_b, lo_b], axis=1))


def prep_inputs(cfg_partial, h, gamma, beta, src, dst):
    """Host-side preprocessing. Returns (cfg, shared_arrays, per_core_arrays)."""
    n = cfg_partial["n_nodes"]
    d = cfg_partial["d"]
    n_cores = cfg_partial["n_cores"]
    split = cfg_partial["split"]
    lo_mode = cfg_partial.get("lo_mode", "fp8")

    src = np.asarray(src).astype(np.int64)
    dst = np.asarray(dst).astype(np.int64)
    h = np.asarray(h, dtype=np.float32)

    n_tiles = math.ceil(n / P)
    nt = math.ceil(n_tiles / n_cores)
    n_tiles_pad = nt * n_cores

    tile_id = dst // P
    local = (dst % P).astype(np.float32)
    is_hi = (src >= split).astype(np.int64)

    order = np.lexsort((src, is_hi, tile_id))
    st = src[order]
    lt = local[order]
    ht = is_hi[order]
    tid = tile_id[order]

    group = tid * 2 + ht
    counts = np.bincount(group, minlength=2 * n_tiles_pad)
    starts = np.zeros(2 * n_tiles_pad + 1, dtype=np.int64)
    np.cumsum(counts, out=starts[1:])
    pos = np.arange(len(st), dtype=np.int64) - np.repeat(starts[:-1], counts)
    counts2 = counts.reshape(n_tiles_pad, 2)

    # Per-core processing order: tiles sorted by descending total count so the
    # rank-r counts are nearly equal across cores; the SPMD program bakes the
    # per-rank max as its valid gather count and -1 pads (skipped by SWDGE)
    # fill the rest of the last chunk.
    tiles = np.arange(n_tiles_pad).reshape(n_cores, nt)
    tot = counts2.sum(1)
    perm = np.stack(
        [tiles[k][np.argsort(-tot[tiles[k]], kind="stable")] for k in range(n_cores)]
    )  # [n_cores, nt]
    lo_common = counts2[perm, 0].max(axis=0)  # [nt]
    hi_common = counts2[perm, 1].max(axis=0)
    c_lo = max(1, int(np.max(np.ceil(lo_common / P))))
    c_hi = max(1, int(np.max(np.ceil(hi_common / P))))

    lo_eff = np.maximum(lo_common, 1)
    hi_eff = np.maximum(hi_common, 1)

    cfg = Cfg(
        n_nodes=n, d=d, n_cores=n_cores, split=split, c_lo=c_lo, c_hi=c_hi,
        lo_mode=lo_mode,
        lo_eff=tuple(int(x) for x in lo_eff),
        hi_eff=tuple(int(x) for x in hi_eff),
    )
    c = cfg.c

    slot = np.where(ht == 1, cfg.c_lo * P + pos, pos)
    rng = np.random.default_rng(1234)
    idx_pad = np.full((n_tiles_pad, c * P), -1, dtype=np.int16)
    dst_pad = np.full((n_tiles_pad, c * P), -1.0, dtype=np.float32)
    idx_rel = (st - ht * split).astype(np.int16)
    idx_pad[tid, slot] = idx_rel
    dst_pad[tid, slot] = lt

    # Common-count pad slots get pseudo-random spread indices (a constant pad
    # index funnels every pad descriptor to one HBM channel; HW-measured 2.5x
    # slow). Slots beyond lo_eff/hi_eff stay -1 and cost no descriptor.
    n_lo, n_hi = split, n - split
    for k in range(n_cores):
        for r in range(nt):
            t = perm[k, r]
            cl = int(counts2[t, 0])
            if lo_eff[r] > cl:
                idx_pad[t, cl : lo_eff[r]] = rng.integers(
                    0, n_lo, lo_eff[r] - cl, dtype=np.int16
                )
            ch = int(counts2[t, 1])
            if hi_eff[r] > ch:
                idx_pad[t, c_lo * P + ch : c_lo * P + hi_eff[r]] = rng.integers(
                    0, n_hi, hi_eff[r] - ch, dtype=np.int16
                )

    h2 = _pack_table(h, lo_mode)

    iota = np.tile(np.arange(P, dtype=np.float32), (P, 1))
    gb = np.concatenate(
        [np.asarray(gamma, np.float32), np.asarray(beta, np.float32)]
    ).reshape(1, 2 * d)

    shared = dict(h2=h2, iota=iota, gb=gb)

    per_core = []
    for k in range(n_cores):
        ip = idx_pad[perm[k]]  # [nt, c*P] int16, processing order
        blk = ip.reshape(nt, c * 8, 16).transpose(0, 2, 1)  # [nt, 16, c*8]
        idx16 = np.tile(blk.transpose(1, 0, 2).reshape(16, nt * c * 8), (8, 1))
        dstv = (
            dst_pad[perm[k]]
            .reshape(nt, c, P)
            .transpose(2, 0, 1)
            .reshape(P, nt * c)
        )
        per_core.append(
            dict(
                idx16=np.ascontiguousarray(idx16),
                dstv=np.ascontiguousarray(dstv),
                tile_order=perm[k].copy(),
            )
        )
    return cfg, shared, per_core


def build_program(cfg: Cfg, repeat_phase1: int = 1, gather_split: int = GSPLIT,
                  g_bufs: int | None = None, nq: int = 4):
    import concourse.bacc as bacc
    import concourse.tile as tile
    from concourse import mybir

    dt = mybir.dt
    d = cfg.d
    nt = cfg.nt
    c_lo, c_hi, c = cfg.c_lo, cfg.c_hi, cfg.c
    rb = cfg.row_bytes  # bytes per table row

    tab_dt = {"fp8": dt.int8, "bf16": dt.bfloat16, "none": dt.bfloat16}[cfg.lo_mode]
    tab_cols = rb // mybir.dt.size(tab_dt)

    # 4 SWDGE queues: gather descriptor processing parallelizes across the
    # gpsimd SWDGE cores (HW-measured 9.1 -> 4.9 ns/row going 1q -> 4q).
    nc = bacc.Bacc(
        "TRN2", target_bir_lowering=False, debug=False, num_devices=cfg.n_cores,
        num_swdge_queues=nq,
    )

    h2_t = nc.dram_tensor("h2", [cfg.n_nodes, tab_cols], tab_dt, kind="ExternalInput")
    idx_t = nc.dram_tensor("idx16", [P, nt * c * 8], dt.int16, kind="ExternalInput")
    dstv_t = nc.dram_tensor("dstv", [P, nt * c], dt.float32, kind="ExternalInput")
    iota_t = nc.dram_tensor("iota", [P, P], dt.float32, kind="ExternalInput")
    gb_t = nc.dram_tensor("gb", [1, 2 * d], dt.float32, kind="ExternalInput")
    out_t = nc.dram_tensor("out", [nt * P, d], dt.float32, kind="ExternalOutput")

    h2_ap = h2_t.ap()
    h2_half = [h2_ap[0 : cfg.split, :], h2_ap[cfg.split : cfg.n_nodes, :]]

    def rhs_views(g, cc):
        """matmul rhs slices (list of (rhs_ap, which_mask)) for chunk cc."""
        row = g[:, cc, :]
        if cfg.lo_mode == "none":
            return [(row, "hi")]
        if cfg.lo_mode == "bf16":
            return [(row[:, 0:d], "hi"), (row[:, d : 2 * d], "hi")]
        return [
            (row[:, 0 : 2 * d].bitcast(dt.bfloat16), "hi"),
            (row[:, 2 * d : 3 * d].bitcast(dt.float8e4), "lo"),
        ]

    with tile.TileContext(nc) as tc, ExitStack() as ctx:
        singles = ctx.enter_context(tc.tile_pool(name="singles", bufs=1))
        if g_bufs is None:
            g_bufs = 3 if rb <= 3 * d else 2
        gpool = ctx.enter_context(tc.tile_pool(name="g", bufs=g_bufs))
        mpool = ctx.enter_context(tc.tile_pool(name="mk", bufs=12))
        spool = ctx.enter_context(tc.tile_pool(name="scr", bufs=3))
        pp = ctx.enter_context(tc.tile_pool(name="ps", bufs=2, space="PSUM"))
        pstat = ctx.enter_context(tc.tile_pool(name="pstat", bufs=1, space="PSUM"))
        dram = ctx.enter_context(tc.tile_pool(name="dram", bufs=2, space="DRAM"))

        idx_sb = singles.tile([P, nt * c * 8], dt.int16)
        nc.sync.dma_start(out=idx_sb[:], in_=idx_t.ap())
        dstv_sb = singles.tile([P, nt * c], dt.float32)
        nc.sync.dma_start(out=dstv_sb[:], in_=dstv_t.ap())
        iota_sb = singles.tile([P, P], dt.float32)
        nc.sync.dma_start(out=iota_sb[:], in_=iota_t.ap())
        gb_sb = singles.tile([1, 2 * d], dt.float32)
        nc.sync.dma_start(out=gb_sb[:], in_=gb_t.ap())

        ones_col = singles.tile([P, 1], dt.float32)
        nc.vector.memset(ones_col[:], 1.0)
        ones_row = singles.tile([1, P], dt.float32)
        nc.vector.memset(ones_row[:], 1.0)
        eps_sb = singles.tile([1, 1], dt.float32)
        nc.vector.memset(eps_sb[:], BN_EPS)

        agg = singles.tile([P, nt * d], dt.float32)
        psum_sum = pstat.tile([1, d], dt.float32)
        psum_sq = pstat.tile([1, d], dt.float32)

        # Tile assigns DMASW sem lanes to Pool-engine DMAs round-robin over 8
        # lanes in emission order, and the ucode locks each lane to the first
        # SWDGE queue that uses it -- so queue choice must be a pure function
        # of the lane. The two collective gpsimd.dma_starts after the gathers
        # are hardwired to queue 0, so their lanes map to 0; the remaining six
        # lanes spread over queues 1-3 (evenly: each lane sees 1/8 of pieces).
        # Staggered For_i loops rotate 5 lanes instead, so the slope
        # diagnostic build stays on queue 0.
        n_pieces_total = sum(
            len(_pieces_cnt(cfg.lo_eff[t], cfg.hi_eff[t], c_lo, gather_split))
            for t in range(nt)
        )
        lane_q = [0] * 8
        if repeat_phase1 == 1 and nq > 1:
            coll_lanes = {n_pieces_total % 8, (n_pieces_total + 1) % 8}
            others = [q % nq for q in range(1, 7)] if nq == 2 else [1, 2, 3, 1, 2, 3][: 6] if nq == 4 else [q % nq for q in range(6)]
            spread = [q if q != 0 or nq == 2 else 1 for q in others]
            for lane in range(8):
                if lane not in coll_lanes:
                    lane_q[lane] = spread.pop(0)
        pool_dma_ctr = [0]

        def next_q():
            q = lane_q[pool_dma_ctr[0] % 8]
            pool_dma_ctr[0] += 1
            return q

        rep_cm = tc.For_i(0, repeat_phase1, 1) if repeat_phase1 > 1 else _nullcm()
        with rep_cm:
          for t in range(nt):
            g = gpool.tile([P, c, tab_cols], tab_dt, tag="g")
            # split each half's gather into <=GSPLIT-chunk pieces: smaller
            # SWDGE ops pipeline desc-gen with the transfer drain. valid counts
            # (num_idxs_reg) stop descriptor generation at the -1 pad tail.
            # pre-zero each half's partial last chunk: the gather stops at
            # `valid` (the -1 pad tail emits no descriptors) but the masked
            # matmuls read all 128 partitions of that chunk. Program-order
            # WAW puts the gather's rows on top of the zeros.
            for eff, base in ((cfg.lo_eff[t], 0), (cfg.hi_eff[t], c_lo)):
                if eff % P:
                    nc.vector.memset(g[:, base + eff // P, :], 0)
            for c0, nck, valid, half in _pieces_cnt(
                cfg.lo_eff[t], cfg.hi_eff[t], c_lo, gather_split
            ):
                nc.gpsimd.dma_gather(
                    g[:, c0 : c0 + nck, :],
                    h2_half[half],
                    idx_sb[:, t * c * 8 + c0 * 8 : t * c * 8 + (c0 + nck) * 8],
                    nck * P,
                    valid,
                    tab_cols,
                    single_packet=False,
                    queue_num=next_q(),
                )
            fp8 = cfg.lo_mode == "fp8"
            ps = pp.tile([P, d], dt.float32, tag="ps")
            if fp8:
                ps_lo = pp.tile([P, d], dt.float32, tag="pslo")
            else:
                ps_lo = None
            chunk_list = list(range(math.ceil(cfg.lo_eff[t] / P))) + list(
                range(c_lo, c_lo + math.ceil(cfg.hi_eff[t] / P))
            )
            n_ch = len(chunk_list)
            for j, cc in enumerate(chunk_list):
                views = rhs_views(g, cc)
                mk_hi = mpool.tile([P, P], dt.bfloat16, tag="mkhi")
                nc.vector.tensor_scalar(
                    out=mk_hi[:],
                    in0=iota_sb[:],
                    scalar1=dstv_sb[:, t * c + cc : t * c + cc + 1],
                    scalar2=None,
                    op0=mybir.AluOpType.is_equal,
                )
                n_to_ps = sum(1 for _, w in views if not (fp8 and w == "lo"))
                j_ps = 0
                for rhs, which in views:
                    # lo plane accumulates in its own PSUM with the SAME bf16
                    # 0/1 mask (mixed-dtype matmul); the 1/LO_SCALE is applied
                    # once per tile at PSUM-combine time.
                    if fp8 and which == "lo":
                        nc.tensor.matmul(
                            ps_lo[:], mk_hi[:], rhs,
                            start=(j == 0), stop=(j == n_ch - 1),
                        )
                    else:
                        nc.tensor.matmul(
                            ps[:], mk_hi[:], rhs,
                            start=(j == 0 and j_ps == 0),
                            stop=(j == n_ch - 1 and j_ps == n_to_ps - 1),
                        )
                        j_ps += 1
            a = agg[:, t * d : (t + 1) * d]
            if fp8:
                lo_sc = spool.tile([P, d], dt.float32, tag="losc")
                nc.vector.tensor_scalar_mul(lo_sc[:], ps_lo[:], 1.0 / LO_SCALE)
                nc.vector.tensor_add(out=a, in0=lo_sc[:], in1=ps[:])
            else:
                nc.scalar.activation(a, ps[:], mybir.ActivationFunctionType.Copy)
            sq = spool.tile([P, d], dt.float32, tag="sq")
            nc.scalar.activation(sq[:], a, mybir.ActivationFunctionType.Square)
            nc.tensor.matmul(
                psum_sum[:], ones_col[:], a, start=(t == 0), stop=(t == nt - 1)
            )
            nc.tensor.matmul(
                psum_sq[:], ones_col[:], sq[:], start=(t == 0), stop=(t == nt - 1)
            )

        # ---- phase 2: global stats + scale/shift --------------------------
        stats = singles.tile([1, 2 * d], dt.float32)
        nc.vector.tensor_copy(out=stats[:, 0:d], in_=psum_sum[:])
        nc.vector.tensor_copy(out=stats[:, d : 2 * d], in_=psum_sq[:])

        cin = dram.tile([1, 2 * d], dt.float32)
        cout = dram.tile([1, 2 * d], dt.float32)
        nc.gpsimd.dma_start(out=cin[:], in_=stats[:])
        nc.gpsimd.collective_compute(
            "AllReduce",
            mybir.AluOpType.add,
            replica_groups=[list(range(cfg.n_cores))],
            ins=[cin.opt()],
            outs=[cout.opt()],
        )
        nc.gpsimd.dma_start(out=stats[:], in_=cout[:])

        inv_n = 1.0 / float(cfg.n_nodes)
        mean = singles.tile([1, d], dt.float32)
        ex2 = singles.tile([1, d], dt.float32)
        nc.vector.tensor_scalar_mul(mean[:], stats[:, 0:d], inv_n)
        nc.vector.tensor_scalar_mul(ex2[:], stats[:, d : 2 * d], inv_n)
        var = singles.tile([1, d], dt.float32)
        nc.vector.tensor_mul(var[:], mean[:], mean[:])
        nc.vector.tensor_tensor(
            out=var[:], in0=ex2[:], in1=var[:], op=mybir.AluOpType.subtract
        )
        rstd = singles.tile([1, d], dt.float32)
        nc.scalar.activation(
            rstd[:],
            var[:],
            mybir.ActivationFunctionType.Sqrt,
            bias=eps_sb[:],
            scale=1.0,
        )
        nc.vector.reciprocal(out=rstd[:], in_=rstd[:])

        scsh = singles.tile([1, 2 * d], dt.float32)
        nc.vector.tensor_mul(scsh[:, 0:d], gb_sb[:, 0:d], rstd[:])  # scale
        tmp = singles.tile([1, d], dt.float32)
        nc.vector.tensor_mul(tmp[:], mean[:], scsh[:, 0:d])
        nc.vector.tensor_tensor(
            out=scsh[:, d : 2 * d],
            in0=gb_sb[:, d : 2 * d],
            in1=tmp[:],
            op=mybir.AluOpType.subtract,
        )

        psb = pstat.tile([P, 2 * d], dt.float32)
        nc.tensor.matmul(psb[:], ones_row[:], scsh[:], start=True, stop=True)
        bc = singles.tile([P, 2 * d], dt.float32)
        nc.vector.tensor_copy(out=bc[:], in_=psb[:])

        # ---- phase 3: normalize + relu + writeback ------------------------
        out_ap = out_t.ap()
        for t in range(nt):
            a = agg[:, t * d : (t + 1) * d]
            y = spool.tile([P, d], dt.float32, tag="y")
            nc.vector.tensor_mul(y[:], a, bc[:, 0:d])
            nc.vector.tensor_add(out=y[:], in0=y[:], in1=bc[:, d : 2 * d])
            nc.vector.tensor_scalar_max(y[:], y[:], 0.0)
            nc.sync.dma_start(out=out_ap[t * P : (t + 1) * P, :], in_=y[:])

    nc.compile()
    return nc


_CACHE: dict = {}


def _get_program(cfg: Cfg):
    # gather_split/g_bufs/nq from the HW sweep: {5,6} x {4 bufs} x {4 queues}
    # is the plateau (~1.3 ms/run vs 2.15 ms for 8/3/1q)
    if cfg not in _CACHE:
        _CACHE[cfg] = build_program(cfg, gather_split=6, g_bufs=4, nq=4)
    return _CACHE[cfg]


def run(cfg: Cfg, shared, per_core, trace=False):
    from concourse.bass_utils import run_bass_kernel_spmd

    nc = _get_program(cfg)
    in_maps = [
        dict(
            h2=shared["h2"],
            idx16=pc["idx16"],
            dstv=pc["dstv"],
            iota=shared["iota"],
            gb=shared["gb"],
        )
        for pc in per_core
    ]
    res = run_bass_kernel_spmd(
        nc, in_maps, core_ids=list(range(cfg.n_cores)), trace=trace
    )
    full = np.empty((cfg.nt * cfg.n_cores * P, cfg.d), np.float32)
    for k, r in enumerate(res.results):
        out_k = r["out"]
        for rank, t in enumerate(per_core[k]["tile_order"]):
            full[t * P : (t + 1) * P] = out_k[rank * P : (rank + 1) * P]
    return full[: cfg.n_nodes], res


def kernel(**inputs) -> np.ndarray:
    h = np.asarray(inputs["h"], dtype=np.float32)
    gamma = np.asarray(inputs["gamma"], dtype=np.float32)
    beta = np.asarray(inputs["beta"], dtype=np.float32)
    src = np.asarray(inputs["src"])
    dst = np.asarray(inputs["dst"])

    n, d = h.shape
    cfg_partial = dict(
        n_nodes=n, d=d, n_cores=8, split=min(n, 25000), lo_mode="none"
    )
    cfg, shared, per_core = prep_inputs(cfg_partial, h, gamma, beta, src, dst)
    full, _ = run(cfg, shared, per_core)
    return full.astype(np.float32)



# revision 25
# speedup vs baseline: 1.4362x; 1.1105x over previous
"""GNN message-passing layer (segment_sum + BatchNorm(train) + ReLU) on 8 Trainium2 cores.

Strategy (dst-sharded, fully local segment sum):
  - Sort edges by (dst_tile, src_half, src). dst tiles are 128-node windows;
    each core owns a fixed set of tiles, so the segment-sum is local to one
    core (no [N,D] all-reduce at all). Each core processes its tiles in
    descending-edge-count order so the rank-r tile's edge count is nearly
    equal across cores; the shared SPMD program bakes the per-rank max as
    its gather count (pads ~2%, with -1 index tails skipped by SWDGE).
  - Per dst tile: bulk-gather h[src] rows via the SWDGE dma_gather custom
    instruction (int16 indices => the node table is split at SPLIT=25000 into
    two <32768-row halves; chunks are homogeneous lo/hi by construction).
    Gathers cost ~9.4 ns/row on one SWDGE queue regardless of source
    (HBM or SBUF) or row bytes -- per-descriptor machinery bound. Spreading
    pieces over 4 SWDGE queues (lane-consistent with Tile's 8 DMASW sem
    lanes) reaches ~4.9 ns/row. Rows are bf16 (512B); hi/lo fp8 packing is
    not worth extra bytes at rel-err tolerance 2e-2 (bf16 gives ~2e-3).
  - Segment sum via per-chunk [128e x 128n] 0/1 masks on the vector engine
    (mask = is_equal(iota_row, dst_local)) feeding PE matmuls that
    accumulate in fp32 PSUM:  agg = sum_e onehot(dst) * h_bf16[src].
  - BatchNorm stats: per-tile ones-vector matmuls accumulate column sums of
    agg and agg^2 in PSUM; a tiny [1,512] AllReduce across the 8 cores gives
    global mean/var; the elementwise chain is local; output rows are written
    per-core and reassembled (tile permutation undone) on the host.
"""

import math
import os
import sys
from contextlib import ExitStack
from dataclasses import dataclass

import numpy as np

try:
    import ml_dtypes
except ImportError:  # pragma: no cover
    ml_dtypes = None

_REPO = "/opt/trn_rl_repo"
if _REPO not in sys.path and os.path.isdir(_REPO):
    sys.path.insert(0, _REPO)

P = 128
BN_EPS = 1e-5
LO_SCALE = 64.0  # lo plane stored as fp8e4m3 * LO_SCALE; mask carries 1/64
GSPLIT = 8  # max chunks per dma_gather piece (desc-gen/transfer pipelining)


def _pieces_cnt(cnt_lo, cnt_hi, c_lo, gsplit=GSPLIT):
    """(chunk0, nchunks, valid_count, half) gather pieces for one tile.

    Chunk columns [0, c_lo) hold lo-half rows, [c_lo, c) hi-half rows.
    valid_count is the number of real (non -1) indices in the piece; the
    remainder of the last chunk is -1 filled and skipped by the gather.
    """
    out = []
    for cnt, base, half in ((cnt_lo, 0, 0), (cnt_hi, c_lo, 1)):
        nch = math.ceil(cnt / P)
        a0 = 0
        while a0 < nch:
            a1 = min(a0 + gsplit, nch)
            valid = min(cnt - a0 * P, (a1 - a0) * P)
            out.append((base + a0, a1 - a0, valid, half))
            a0 = a1
    return out


class _nullcm:
    def __enter__(self):
        return None

    def __exit__(self, *a):
        return False


@dataclass(frozen=True)
class Cfg:
    n_nodes: int
    d: int
    n_cores: int
    split: int
    c_lo: int
    c_hi: int
    lo_mode: str = "fp8"  # "fp8" | "bf16" | "none"
    # per tile-rank valid gather counts (common across cores; tiles are
    # processed in per-core descending-count order so ranks align)
    lo_eff: tuple = ()
    hi_eff: tuple = ()

    @property
    def n_tiles(self) -> int:
        return math.ceil(self.n_nodes / P)

    @property
    def nt(self) -> int:  # tiles per core
        return math.ceil(self.n_tiles / self.n_cores)

    @property
    def c(self) -> int:
        return self.c_lo + self.c_hi

    @property
    def row_bytes(self) -> int:  # gathered bytes per node row
        return {"fp8": 3 * self.d, "bf16": 4 * self.d, "none": 2 * self.d}[
            self.lo_mode
        ]

    @property
    def g_bufs(self) -> int:
        return 3 if self.row_bytes <= 3 * self.d else 2


def _bf16(x):
    return x.astype(ml_dtypes.bfloat16)


def _pack_table(h, lo_mode):
    """Build the gather table. Returns (array, np_dtype_name)."""
    hi = _bf16(h)
    if lo_mode == "none":
        return np.ascontiguousarray(hi)
    lo = h - hi.astype(np.float32)
    if lo_mode == "bf16":
        return np.ascontiguousarray(np.concatenate([hi, _bf16(lo)], axis=1))
    # fp8: [hi bf16 bytes | fp8(lo*64) bytes] as one int8 row
    lo8 = (lo * LO_SCALE).astype(ml_dtypes.float8_e4m3)
    hi_b = hi.view(np.int8)  # [N, 2D]
    lo_b = lo8.view(np.int8)  # [N, D]
    return np.ascontiguousarray(np.concatenate([hi_b, lo_b], axis=1))


def prep_inputs(cfg_partial, h, gamma, beta, src, dst):
    """Host-side preprocessing. Returns (cfg, shared_arrays, per_core_arrays)."""
    n = cfg_partial["n_nodes"]
    d = cfg_partial["d"]
    n_cores = cfg_partial["n_cores"]
    split = cfg_partial["split"]
    lo_mode = cfg_partial.get("lo_mode", "fp8")

    src = np.asarray(src).astype(np.int64)
    dst = np.asarray(dst).astype(np.int64)
    h = np.asarray(h, dtype=np.float32)

    n_tiles = math.ceil(n / P)
    nt = math.ceil(n_tiles / n_cores)
    n_tiles_pad = nt * n_cores

    tile_id = dst // P
    local = (dst % P).astype(np.float32)
    is_hi = (src >= split).astype(np.int64)

    order = np.lexsort((src, is_hi, tile_id))
    st = src[order]
    lt = local[order]
    ht = is_hi[order]
    tid = tile_id[order]

    group = tid * 2 + ht
    counts = np.bincount(group, minlength=2 * n_tiles_pad)
    starts = np.zeros(2 * n_tiles_pad + 1, dtype=np.int64)
    np.cumsum(counts, out=starts[1:])
    pos = np.arange(len(st), dtype=np.int64) - np.repeat(starts[:-1], counts)
    counts2 = counts.reshape(n_tiles_pad, 2)

    # Per-core processing order: tiles sorted by descending total count so the
    # rank-r counts are nearly equal across cores; the SPMD program bakes the
    # per-rank max as its valid gather count and -1 pads (skipped by SWDGE)
    # fill the rest of the last chunk.
    tiles = np.arange(n_tiles_pad).reshape(n_cores, nt)
    tot = counts2.sum(1)
    perm = np.stack(
        [tiles[k][np.argsort(-tot[tiles[k]], kind="stable")] for k in range(n_cores)]
    )  # [n_cores, nt]
    lo_common = counts2[perm, 0].max(axis=0)  # [nt]
    hi_common = counts2[perm, 1].max(axis=0)
    c_lo = max(1, int(np.max(np.ceil(lo_common / P))))
    c_hi = max(1, int(np.max(np.ceil(hi_common / P))))

    lo_eff = np.maximum(lo_common, 1)
    hi_eff = np.maximum(hi_common, 1)
    if cfg_partial.get("pad_full"):
        # no -1 descriptor-skip tails: every chunk fully gathered (the skip
        # measured time-neutral; this removes the per-tile memset WAW edge)
        lo_eff = np.ceil(lo_eff / P).astype(np.int64) * P
        hi_eff = np.ceil(hi_eff / P).astype(np.int64) * P

    cfg = Cfg(
        n_nodes=n, d=d, n_cores=n_cores, split=split, c_lo=c_lo, c_hi=c_hi,
        lo_mode=lo_mode,
        lo_eff=tuple(int(x) for x in lo_eff),
        hi_eff=tuple(int(x) for x in hi_eff),
    )
    c = cfg.c

    slot = np.where(ht == 1, cfg.c_lo * P + pos, pos)
    rng = np.random.default_rng(1234)
    idx_pad = np.full((n_tiles_pad, c * P), -1, dtype=np.int16)
    dst_pad = np.full((n_tiles_pad, c * P), -1.0, dtype=np.float32)
    idx_rel = (st - ht * split).astype(np.int16)
    idx_pad[tid, slot] = idx_rel
    dst_pad[tid, slot] = lt

    # Common-count pad slots get pseudo-random spread indices (a constant pad
    # index funnels every pad descriptor to one HBM channel; HW-measured 2.5x
    # slow). Slots beyond lo_eff/hi_eff stay -1 and cost no descriptor.
    n_lo, n_hi = split, n - split
    for k in range(n_cores):
        for r in range(nt):
            t = perm[k, r]
            cl = int(counts2[t, 0])
            if lo_eff[r] > cl:
                idx_pad[t, cl : lo_eff[r]] = rng.integers(
                    0, n_lo, lo_eff[r] - cl, dtype=np.int16
                )
            ch = int(counts2[t, 1])
            if hi_eff[r] > ch:
                idx_pad[t, c_lo * P + ch : c_lo * P + hi_eff[r]] = rng.integers(
                    0, n_hi, hi_eff[r] - ch, dtype=np.int16
                )

    h2 = _pack_table(h, lo_mode)

    iota = np.tile(np.arange(P, dtype=np.float32), (P, 1))
    gb = np.concatenate(
        [np.asarray(gamma, np.float32), np.asarray(beta, np.float32)]
    ).reshape(1, 2 * d)

    shared = dict(h2=h2, iota=iota, gb=gb)

    per_core = []
    for k in range(n_cores):
        ip = idx_pad[perm[k]]  # [nt, c*P] int16, processing order
        blk = ip.reshape(nt, c * 8, 16).transpose(0, 2, 1)  # [nt, 16, c*8]
        idx16 = np.tile(blk.transpose(1, 0, 2).reshape(16, nt * c * 8), (8, 1))
        dstv = (
            dst_pad[perm[k]]
            .reshape(nt, c, P)
            .transpose(2, 0, 1)
            .reshape(P, nt * c)
        )
        per_core.append(
            dict(
                idx16=np.ascontiguousarray(idx16),
                dstv=np.ascontiguousarray(dstv),
                tile_order=perm[k].copy(),
            )
        )
    return cfg, shared, per_core


def build_program(cfg: Cfg, repeat_phase1: int = 1, gather_split: int = GSPLIT,
                  g_bufs: int | None = None, nq: int = 4):
    import concourse.bacc as bacc
    import concourse.tile as tile
    from concourse import mybir

    dt = mybir.dt
    d = cfg.d
    nt = cfg.nt
    c_lo, c_hi, c = cfg.c_lo, cfg.c_hi, cfg.c
    rb = cfg.row_bytes  # bytes per table row

    tab_dt = {"fp8": dt.int8, "bf16": dt.bfloat16, "none": dt.bfloat16}[cfg.lo_mode]
    tab_cols = rb // mybir.dt.size(tab_dt)

    # 4 SWDGE queues: gather descriptor processing parallelizes across the
    # gpsimd SWDGE cores (HW-measured 9.1 -> 4.9 ns/row going 1q -> 4q).
    nc = bacc.Bacc(
        "TRN2", target_bir_lowering=False, debug=False, num_devices=cfg.n_cores,
        num_swdge_queues=nq,
    )

    h2_t = nc.dram_tensor("h2", [cfg.n_nodes, tab_cols], tab_dt, kind="ExternalInput")
    idx_t = nc.dram_tensor("idx16", [P, nt * c * 8], dt.int16, kind="ExternalInput")
    dstv_t = nc.dram_tensor("dstv", [P, nt * c], dt.float32, kind="ExternalInput")
    iota_t = nc.dram_tensor("iota", [P, P], dt.float32, kind="ExternalInput")
    gb_t = nc.dram_tensor("gb", [1, 2 * d], dt.float32, kind="ExternalInput")
    out_t = nc.dram_tensor("out", [nt * P, d], dt.float32, kind="ExternalOutput")

    h2_ap = h2_t.ap()
    h2_half = [h2_ap[0 : cfg.split, :], h2_ap[cfg.split : cfg.n_nodes, :]]

    def rhs_views(g, cc):
        """matmul rhs slices (list of (rhs_ap, which_mask)) for chunk cc."""
        row = g[:, cc, :]
        if cfg.lo_mode == "none":
            return [(row, "hi")]
        if cfg.lo_mode == "bf16":
            return [(row[:, 0:d], "hi"), (row[:, d : 2 * d], "hi")]
        return [
            (row[:, 0 : 2 * d].bitcast(dt.bfloat16), "hi"),
            (row[:, 2 * d : 3 * d].bitcast(dt.float8e4), "lo"),
        ]

    with tile.TileContext(nc) as tc, ExitStack() as ctx:
        singles = ctx.enter_context(tc.tile_pool(name="singles", bufs=1))
        if g_bufs is None:
            g_bufs = 3 if rb <= 3 * d else 2
        gpool = ctx.enter_context(tc.tile_pool(name="g", bufs=g_bufs))
        mpool = ctx.enter_context(tc.tile_pool(name="mk", bufs=12))
        spool = ctx.enter_context(tc.tile_pool(name="scr", bufs=3))
        pp = ctx.enter_context(tc.tile_pool(name="ps", bufs=2, space="PSUM"))
        pstat = ctx.enter_context(tc.tile_pool(name="pstat", bufs=1, space="PSUM"))
        dram = ctx.enter_context(tc.tile_pool(name="dram", bufs=2, space="DRAM"))

        idx_sb = singles.tile([P, nt * c * 8], dt.int16)
        nc.sync.dma_start(out=idx_sb[:], in_=idx_t.ap())
        dstv_sb = singles.tile([P, nt * c], dt.float32)
        nc.sync.dma_start(out=dstv_sb[:], in_=dstv_t.ap())
        iota_sb = singles.tile([P, P], dt.float32)
        nc.sync.dma_start(out=iota_sb[:], in_=iota_t.ap())
        gb_sb = singles.tile([1, 2 * d], dt.float32)
        nc.sync.dma_start(out=gb_sb[:], in_=gb_t.ap())

        ones_col = singles.tile([P, 1], dt.float32)
        nc.vector.memset(ones_col[:], 1.0)
        ones_row = singles.tile([1, P], dt.float32)
        nc.vector.memset(ones_row[:], 1.0)
        eps_sb = singles.tile([1, 1], dt.float32)
        nc.vector.memset(eps_sb[:], BN_EPS)

        agg = singles.tile([P, nt * d], dt.float32)
        psum_sum = pstat.tile([1, d], dt.float32)
        psum_sq = pstat.tile([1, d], dt.float32)

        # Tile assigns DMASW sem lanes to Pool-engine DMAs round-robin over 8
        # lanes in emission order, and the ucode locks each lane to the first
        # SWDGE queue that uses it -- so queue choice must be a pure function
        # of the lane. The two collective gpsimd.dma_starts after the gathers
        # are hardwired to queue 0, so their lanes map to 0; the remaining six
        # lanes spread over queues 1-3 (evenly: each lane sees 1/8 of pieces).
        # Staggered For_i loops rotate 5 lanes instead, so the slope
        # diagnostic build stays on queue 0.
        n_pieces_total = sum(
            len(_pieces_cnt(cfg.lo_eff[t], cfg.hi_eff[t], c_lo, gather_split))
            for t in range(nt)
        )
        lane_q = [0] * 8
        if repeat_phase1 == 1 and nq > 1:
            coll_lanes = {n_pieces_total % 8, (n_pieces_total + 1) % 8}
            others = [q % nq for q in range(1, 7)] if nq == 2 else [1, 2, 3, 1, 2, 3][: 6] if nq == 4 else [q % nq for q in range(6)]
            spread = [q if q != 0 or nq == 2 else 1 for q in others]
            for lane in range(8):
                if lane not in coll_lanes:
                    lane_q[lane] = spread.pop(0)
        pool_dma_ctr = [0]

        def next_q():
            q = lane_q[pool_dma_ctr[0] % 8]
            pool_dma_ctr[0] += 1
            return q

        rep_cm = tc.For_i(0, repeat_phase1, 1) if repeat_phase1 > 1 else _nullcm()
        with rep_cm:
          for t in range(nt):
            g = gpool.tile([P, c, tab_cols], tab_dt, tag="g")
            # split each half's gather into <=GSPLIT-chunk pieces: smaller
            # SWDGE ops pipeline desc-gen with the transfer drain. valid counts
            # (num_idxs_reg) stop descriptor generation at the -1 pad tail.
            # pre-zero each half's partial last chunk: the gather stops at
            # `valid` (the -1 pad tail emits no descriptors) but the masked
            # matmuls read all 128 partitions of that chunk. Program-order
            # WAW puts the gather's rows on top of the zeros.
            for eff, base in ((cfg.lo_eff[t], 0), (cfg.hi_eff[t], c_lo)):
                if eff % P:
                    nc.vector.memset(g[:, base + eff // P, :], 0)
            for c0, nck, valid, half in _pieces_cnt(
                cfg.lo_eff[t], cfg.hi_eff[t], c_lo, gather_split
            ):
                nc.gpsimd.dma_gather(
                    g[:, c0 : c0 + nck, :],
                    h2_half[half],
                    idx_sb[:, t * c * 8 + c0 * 8 : t * c * 8 + (c0 + nck) * 8],
                    nck * P,
                    valid,
                    tab_cols,
                    single_packet=False,
                    queue_num=next_q(),
                )
            fp8 = cfg.lo_mode == "fp8"
            ps = pp.tile([P, d], dt.float32, tag="ps")
            if fp8:
                ps_lo = pp.tile([P, d], dt.float32, tag="pslo")
            else:
                ps_lo = None
            chunk_list = list(range(math.ceil(cfg.lo_eff[t] / P))) + list(
                range(c_lo, c_lo + math.ceil(cfg.hi_eff[t] / P))
            )
            n_ch = len(chunk_list)
            for j, cc in enumerate(chunk_list):
                views = rhs_views(g, cc)
                mk_hi = mpool.tile([P, P], dt.bfloat16, tag="mkhi")
                nc.vector.tensor_scalar(
                    out=mk_hi[:],
                    in0=iota_sb[:],
                    scalar1=dstv_sb[:, t * c + cc : t * c + cc + 1],
                    scalar2=None,
                    op0=mybir.AluOpType.is_equal,
                )
                n_to_ps = sum(1 for _, w in views if not (fp8 and w == "lo"))
                j_ps = 0
                for rhs, which in views:
                    # lo plane accumulates in its own PSUM with the SAME bf16
                    # 0/1 mask (mixed-dtype matmul); the 1/LO_SCALE is applied
                    # once per tile at PSUM-combine time.
                    if fp8 and which == "lo":
                        nc.tensor.matmul(
                            ps_lo[:], mk_hi[:], rhs,
                            start=(j == 0), stop=(j == n_ch - 1),
                        )
                    else:
                        nc.tensor.matmul(
                            ps[:], mk_hi[:], rhs,
                            start=(j == 0 and j_ps == 0),
                            stop=(j == n_ch - 1 and j_ps == n_to_ps - 1),
                        )
                        j_ps += 1
            a = agg[:, t * d : (t + 1) * d]
            if fp8:
                lo_sc = spool.tile([P, d], dt.float32, tag="losc")
                nc.vector.tensor_scalar_mul(lo_sc[:], ps_lo[:], 1.0 / LO_SCALE)
                nc.vector.tensor_add(out=a, in0=lo_sc[:], in1=ps[:])
            else:
                nc.scalar.activation(a, ps[:], mybir.ActivationFunctionType.Copy)
            sq = spool.tile([P, d], dt.float32, tag="sq")
            nc.scalar.activation(sq[:], a, mybir.ActivationFunctionType.Square)
            nc.tensor.matmul(
                psum_sum[:], ones_col[:], a, start=(t == 0), stop=(t == nt - 1)
            )
            nc.tensor.matmul(
                psum_sq[:], ones_col[:], sq[:], start=(t == 0), stop=(t == nt - 1)
            )

        # ---- phase 2: global stats + scale/shift --------------------------
        stats = singles.tile([1, 2 * d], dt.float32)
        nc.vector.tensor_copy(out=stats[:, 0:d], in_=psum_sum[:])
        nc.vector.tensor_copy(out=stats[:, d : 2 * d], in_=psum_sq[:])

        cin = dram.tile([1, 2 * d], dt.float32)
        cout = dram.tile([1, 2 * d], dt.float32)
        nc.gpsimd.dma_start(out=cin[:], in_=stats[:])
        nc.gpsimd.collective_compute(
            "AllReduce",
            mybir.AluOpType.add,
            replica_groups=[list(range(cfg.n_cores))],
            ins=[cin.opt()],
            outs=[cout.opt()],
        )
        nc.gpsimd.dma_start(out=stats[:], in_=cout[:])

        inv_n = 1.0 / float(cfg.n_nodes)
        mean = singles.tile([1, d], dt.float32)
        ex2 = singles.tile([1, d], dt.float32)
        nc.vector.tensor_scalar_mul(mean[:], stats[:, 0:d], inv_n)
        nc.vector.tensor_scalar_mul(ex2[:], stats[:, d : 2 * d], inv_n)
        var = singles.tile([1, d], dt.float32)
        nc.vector.tensor_mul(var[:], mean[:], mean[:])
        nc.vector.tensor_tensor(
            out=var[:], in0=ex2[:], in1=var[:], op=mybir.AluOpType.subtract
        )
        rstd = singles.tile([1, d], dt.float32)
        nc.scalar.activation(
            rstd[:],
            var[:],
            mybir.ActivationFunctionType.Sqrt,
            bias=eps_sb[:],
            scale=1.0,
        )
        nc.vector.reciprocal(out=rstd[:], in_=rstd[:])

        scsh = singles.tile([1, 2 * d], dt.float32)
        nc.vector.tensor_mul(scsh[:, 0:d], gb_sb[:, 0:d], rstd[:])  # scale
        tmp = singles.tile([1, d], dt.float32)
        nc.vector.tensor_mul(tmp[:], mean[:], scsh[:, 0:d])
        nc.vector.tensor_tensor(
            out=scsh[:, d : 2 * d],
            in0=gb_sb[:, d : 2 * d],
            in1=tmp[:],
            op=mybir.AluOpType.subtract,
        )

        psb = pstat.tile([P, 2 * d], dt.float32)
        nc.tensor.matmul(psb[:], ones_row[:], scsh[:], start=True, stop=True)
        bc = singles.tile([P, 2 * d], dt.float32)
        nc.vector.tensor_copy(out=bc[:], in_=psb[:])

        # ---- phase 3: normalize + relu + writeback ------------------------
        out_ap = out_t.ap()
        for t in range(nt):
            a = agg[:, t * d : (t + 1) * d]
            y = spool.tile([P, d], dt.float32, tag="y")
            nc.vector.tensor_mul(y[:], a, bc[:, 0:d])
            nc.vector.tensor_add(out=y[:], in0=y[:], in1=bc[:, d : 2 * d])
            nc.vector.tensor_scalar_max(y[:], y[:], 0.0)
            nc.sync.dma_start(out=out_ap[t * P : (t + 1) * P, :], in_=y[:])

    nc.compile()
    return nc


_CACHE: dict = {}


def _get_program(cfg: Cfg):
    # gather_split/g_bufs/nq from the HW sweep: {5,6} x {4 bufs} x {4 queues}
    # is the plateau (~1.3 ms/run vs 2.15 ms for 8/3/1q)
    if cfg not in _CACHE:
        _CACHE[cfg] = build_program(cfg, gather_split=6, g_bufs=4, nq=4)
    return _CACHE[cfg]


def run(cfg: Cfg, shared, per_core, trace=False):
    from concourse.bass_utils import run_bass_kernel_spmd

    nc = _get_program(cfg)
    in_maps = [
        dict(
            h2=shared["h2"],
            idx16=pc["idx16"],
            dstv=pc["dstv"],
            iota=shared["iota"],
            gb=shared["gb"],
        )
        for pc in per_core
    ]
    res = run_bass_kernel_spmd(
        nc, in_maps, core_ids=list(range(cfg.n_cores)), trace=trace
    )
    full = np.empty((cfg.nt * cfg.n_cores * P, cfg.d), np.float32)
    for k, r in enumerate(res.results):
        out_k = r["out"]
        for rank, t in enumerate(per_core[k]["tile_order"]):
            full[t * P : (t + 1) * P] = out_k[rank * P : (rank + 1) * P]
    return full[: cfg.n_nodes], res


def kernel(**inputs) -> np.ndarray:
    h = np.asarray(inputs["h"], dtype=np.float32)
    gamma = np.asarray(inputs["gamma"], dtype=np.float32)
    beta = np.asarray(inputs["beta"], dtype=np.float32)
    src = np.asarray(inputs["src"])
    dst = np.asarray(inputs["dst"])

    n, d = h.shape
    cfg_partial = dict(
        n_nodes=n, d=d, n_cores=8, split=min(n, 25000), lo_mode="none"
    )
    cfg, shared, per_core = prep_inputs(cfg_partial, h, gamma, beta, src, dst)
    full, _ = run(cfg, shared, per_core)
    return full.astype(np.float32)

